# revision 5
# baseline (speedup 1.0000x reference)
"""Trainium2 Bass kernel for nn_Build_Simulator (Dirichlet-multinomial
subsampled single-cell sum -> log1p -> LayerNorm -> MinMax).

Contract: kernel(**inputs) takes the FULL unsharded inputs (numpy arrays,
keyed as in setup_inputs()) and returns the FULL [18000] float32 output.

Strategy
--------
Host (tiny, O(C*N + K*G) work):
  * Replicate the reference's jax PRNG chain bit-exactly on CPU to get the
    per-celltype 0/1 row masks w[C, N] (sum(w) == 500 selected rows).
  * The masked matvec  total[g] = sum_{c,n} w[c,n] * scdata[c,n,g]  only
    touches the ~500 selected rows, so gather those rows and shard them
    along the gene axis across the 8 NeuronCores (sharding_hint).
Device (8 cores, SPMD, one Bass/Tile program):
  * Each core: DMA its [R, G/8] row shard, reduce over rows on the tensor
    engine (weight column x row tile -> PSUM accumulate), z = ln(total+1)
    on the scalar engine, AllGather the (sum, sumsq) LayerNorm stats,
    normalize (+ gamma/beta if nontrivial), AllGather global (min, max),
    then the final minmax affine, DMA out the [G/8] shard.
Host: concatenate the 8 shards.
"""

import os
import numpy as np

_C, _N, _G = 10, 1000, 18000
_M = 8  # cores
_GS = _G // _M  # genes per core
_TOTAL_COUNT = 500
_LN_EPS = 1e-3
_ALPHA_EPS = 1e-6

# test.py introspection: last BassKernelResults (exec_time_ns when traced)
LAST_RESULTS = None

_PROGRAM_CACHE = {}


def _selection_weights(x, W, b, dtype):
    """Bit-exact CPU replication of the reference's sampling chain."""
    import jax
    import jax.numpy as jnp

    cpu = jax.devices("cpu")[0]
    with jax.default_device(cpu):
        x = jax.device_put(np.asarray(x), cpu)
        W = jax.device_put(np.asarray(W), cpu)
        b = jax.device_put(np.asarray(b), cpu)

        key = jax.random.key(42)
        k_dir, k_sub = jax.random.split(key)

        alpha = jax.nn.relu(x @ W + b) + _ALPHA_EPS  # [B, 10]

        kg, kc = jax.random.split(k_dir)
        g = jax.random.gamma(kg, alpha)
        p = g / jnp.sum(g, axis=-1, keepdims=True)
        logits = jnp.log(p)
        draws = jax.random.categorical(
            kc, logits, shape=(_TOTAL_COUNT,) + alpha.shape[:1]
        )
        counts = jnp.sum(jax.nn.one_hot(draws, alpha.shape[-1], dtype=jnp.int32), axis=0)
        counts0 = counts[0]

        C, N = _C, _N
        keys = jax.random.split(k_sub, C)

        def subsample_weights(key_c, k_c):
            perm = jax.random.permutation(key_c, N)
            mask = (jnp.arange(N) < k_c).astype(dtype)
            return jnp.zeros((N,), dtype=dtype).at[perm].set(mask)

        w = jax.vmap(subsample_weights)(keys, counts0)  # [C, N]
        return np.asarray(w)


def _build_program(R, GS, apply_gb):
    """One SPMD Bass/Tile program: weighted row-sum -> log1p -> LN -> MinMax.

    R: number of (padded) gathered rows, multiple of 128.
    GS: genes per core.
    apply_gb: emit the per-gene gamma/beta multiply-add.
    """
    from concourse import bacc, mybir, tile

    f32 = mybir.dt.float32
    OP = mybir.AluOpType
    X = mybir.AxisListType.X
    ACT = mybir.ActivationFunctionType
    KT = R // 128
    RG = [list(range(_M))]

    nc = bacc.Bacc("TRN2", target_bir_lowering=False, debug=False, num_devices=_M)

    rows_d = nc.dram_tensor("rows", [R, GS], f32, kind="ExternalInput")
    wvec_d = nc.dram_tensor("wvec", [R], f32, kind="ExternalInput")
    if apply_gb:
        gamma_d = nc.dram_tensor("gamma_s", [GS], f32, kind="ExternalInput")
        beta_d = nc.dram_tensor("beta_s", [GS], f32, kind="ExternalInput")
    out_d = nc.dram_tensor("out", [GS], f32, kind="ExternalOutput")

    BLK = 512  # PSUM bank (f32) / max moving free dim
    blocks = [(g0, min(BLK, GS - g0)) for g0 in range(0, GS, BLK)]

    with tile.TileContext(nc) as tc:
        with (
            tc.tile_pool(name="load", bufs=KT) as loadp,
            tc.tile_pool(name="vecs", bufs=1) as vecp,
            tc.tile_pool(name="small", bufs=1) as smallp,
            tc.tile_pool(name="psum", bufs=1, space="PSUM") as psump,
            tc.tile_pool(name="dram", bufs=1, space="DRAM") as dramp,
        ):
            # --- weighted row-sum over the R gathered rows -> PSUM [1, GS]
            wv = smallp.tile([128, KT], f32)
            nc.sync.dma_start(wv[:], wvec_d[:].rearrange("(k p) -> p k", p=128))

            ktiles = []
            for ki in range(KT):
                t = loadp.tile([128, GS], f32, tag="rows")
                nc.sync.dma_start(t[:], rows_d[ki * 128 : (ki + 1) * 128, :])
                ktiles.append(t)

            total_ps = psump.tile([1, GS], f32)
            for ki in range(KT):
                for g0, gsz in blocks:
                    nc.tensor.matmul(
                        total_ps[0:1, g0 : g0 + gsz],
                        wv[:, ki : ki + 1],
                        ktiles[ki][:, g0 : g0 + gsz],
                        start=(ki == 0),
                        stop=(ki == KT - 1),
                    )

            # --- z = ln(total + 1), with free-axis sum accumulated for LN
            z = vecp.tile([1, GS], f32)
            zsq = vecp.tile([1, GS], f32)
            stat = smallp.tile([1, 8], f32)
            nc.vector.memset(stat[:], 0.0)
            nc.scalar.activation(
                z[:], total_ps[0:1, :], ACT.Ln, bias=1.0, scale=1.0,
                accum_out=stat[0:1, 0:1],
            )
            nc.scalar.activation(
                zsq[:], z[:], ACT.Square, accum_out=stat[0:1, 1:2]
            )

            # --- AllGather (sum, sumsq) and reduce across ranks
            st_in = dramp.tile([1, 8], f32)
            st_out = dramp.tile([_M, 8], f32)
            nc.sync.dma_start(st_in[:], stat[:])
            nc.gpsimd.collective_compute(
                "AllGather", OP.bypass, replica_groups=RG,
                ins=[st_in.opt()], outs=[st_out.opt()],
            )
            gst = smallp.tile([1, _M * 8], f32)
            nc.sync.dma_start(gst[:], st_out[:])
            gview = gst[0:1, :].rearrange("p (r v) -> p v r", v=8)  # [1, 8, _M]
            sums = smallp.tile([1, 8], f32)
            nc.vector.tensor_reduce(sums[:], gview, X, OP.add)

            # --- LN scalars: mean, inv = 1/sqrt(var + eps)
            mean = smallp.tile([1, 1], f32)
            msq = smallp.tile([1, 1], f32)
            var = smallp.tile([1, 1], f32)
            vpe = smallp.tile([1, 1], f32)
            sq = smallp.tile([1, 1], f32)
            inv = smallp.tile([1, 1], f32)
            nc.vector.tensor_scalar_mul(mean[:], sums[0:1, 0:1], 1.0 / _G)
            nc.vector.tensor_mul(msq[:], mean[:], mean[:])
            nc.vector.scalar_tensor_tensor(
                var[:], sums[0:1, 1:2], 1.0 / _G, msq[:], OP.mult, OP.subtract
            )
            nc.vector.tensor_scalar_add(vpe[:], var[:], _LN_EPS)
            nc.scalar.activation(sq[:], vpe[:], ACT.Sqrt)
            nc.vector.reciprocal(inv[:], sq[:])

            # --- z_n = (z - mean) * inv  [+ gamma/beta]
            zg = vecp.tile([1, GS], f32)
            nc.vector.tensor_scalar(
                zg[:], z[:], mean[0:1, 0:1], inv[0:1, 0:1], OP.subtract, OP.mult
            )
            if apply_gb:
                gam = vecp.tile([1, GS], f32)
                bet = vecp.tile([1, GS], f32)
                nc.sync.dma_start(gam[:], gamma_d[None, :])
                nc.sync.dma_start(bet[:], beta_d[None, :])
                nc.vector.tensor_mul(zg[:], zg[:], gam[:])
                nc.vector.tensor_add(zg[:], zg[:], bet[:])

            # --- AllGather local (min, max) of z_n and reduce across ranks
            stat2 = smallp.tile([1, 8], f32)
            nc.vector.memset(stat2[:], 0.0)
            nc.vector.tensor_reduce(stat2[0:1, 0:1], zg[:], X, OP.min)
            nc.vector.tensor_reduce(stat2[0:1, 1:2], zg[:], X, OP.max)
            st2_in = dramp.tile([1, 8], f32)
            st2_out = dramp.tile([_M, 8], f32)
            nc.sync.dma_start(st2_in[:], stat2[:])
            nc.gpsimd.collective_compute(
                "AllGather", OP.bypass, replica_groups=RG,
                ins=[st2_in.opt()], outs=[st2_out.opt()],
            )
            gst2 = smallp.tile([1, _M * 8], f32)
            nc.sync.dma_start(gst2[:], st2_out[:])
            g2view = gst2[0:1, :].rearrange("p (r v) -> p v r", v=8)
            mins = smallp.tile([1, 8], f32)
            maxs = smallp.tile([1, 8], f32)
            nc.vector.tensor_reduce(mins[:], g2view, X, OP.min)
            nc.vector.tensor_reduce(maxs[:], g2view, X, OP.max)

            # --- out = (z_n - lo) / (hi - lo)
            den = smallp.tile([1, 1], f32)
            rec = smallp.tile([1, 1], f32)
            nc.vector.tensor_sub(den[:], maxs[0:1, 1:2], mins[0:1, 0:1])
            nc.vector.reciprocal(rec[:], den[:])
            outv = vecp.tile([1, GS], f32)
            nc.vector.tensor_scalar(
                outv[:], zg[:], mins[0:1, 0:1], rec[0:1, 0:1],
                OP.subtract, OP.mult,
            )
            nc.sync.dma_start(out_d[None, :], outv[:])

    nc.compile()
    return nc


def _get_program(R, GS, apply_gb):
    key = (R, GS, apply_gb)
    if key not in _PROGRAM_CACHE:
        _PROGRAM_CACHE[key] = _build_program(R, GS, apply_gb)
    return _PROGRAM_CACHE[key]


def _install_trace_shims():
    """Make trace=True work in this image: provide the missing
    antenv.axon_hooks module (via the boot's ctypes NTFF hook) and stub
    the artifact upload (no bucket access here). Test-only path."""
    import sys
    import types

    try:
        import antenv.axon_hooks  # noqa: F401
    except ImportError:
        mod = types.ModuleType("antenv.axon_hooks")
        mod._hook = None

        def set_axon_ntff_profile_hook(h):
            mod._hook = h

        def get_axon_ntff_profile_hook():
            if mod._hook is None:
                try:
                    from trn_agent_boot.trn_boot import _ntff_profile_via_ctypes

                    mod._hook = _ntff_profile_via_ctypes("/opt/axon/libaxon_pjrt.so")
                except Exception:
                    return None
            return mod._hook

        mod.set_axon_ntff_profile_hook = set_axon_ntff_profile_hook
        mod.get_axon_ntff_profile_hook = get_axon_ntff_profile_hook
        sys.modules["antenv.axon_hooks"] = mod
        import antenv

        antenv.axon_hooks = mod

    from concourse import bass_utils

    bass_utils.upload_artifacts = lambda tmpdir: f"local://{tmpdir}"


def kernel(x, W, b, scdata, gamma, beta):
    global LAST_RESULTS
    from concourse.bass_utils import run_bass_kernel_spmd

    scdata = np.ascontiguousarray(np.asarray(scdata, dtype=np.float32))
    gamma = np.asarray(gamma, dtype=np.float32)
    beta = np.asarray(beta, dtype=np.float32)
    C, N, G = scdata.shape
    assert (C, N, G) == (_C, _N, _G), f"unexpected scdata shape {scdata.shape}"

    # host: sampling chain -> selected rows (c-major order, matching einsum)
    w = _selection_weights(x, W, b, np.float32)  # [C, N] of 0/1
    sel = np.flatnonzero(w.reshape(-1) > 0)
    K = sel.size
    R = max(128, ((K + 127) // 128) * 128)

    rows = np.zeros((R, G), dtype=np.float32)
    rows[:K] = scdata.reshape(C * N, G)[sel]
    wvec = np.zeros((R,), dtype=np.float32)
    wvec[:K] = w.reshape(-1)[sel]  # == 1.0, but stay general

    apply_gb = not (
        np.all(gamma == np.float32(1.0)) and np.all(beta == np.float32(0.0))
    )

    nc = _get_program(R, _GS, apply_gb)

    in_maps = []
    for i in range(_M):
        m = {
            "rows": np.ascontiguousarray(rows[:, i * _GS : (i + 1) * _GS]),
            "wvec": wvec,
        }
        if apply_gb:
            m["gamma_s"] = np.ascontiguousarray(gamma[i * _GS : (i + 1) * _GS])
            m["beta_s"] = np.ascontiguousarray(beta[i * _GS : (i + 1) * _GS])
        in_maps.append(m)

    trace = bool(int(os.environ.get("KERNEL_TRACE", "0")))
    if trace:
        _install_trace_shims()
    res = run_bass_kernel_spmd(
        nc, in_maps, core_ids=list(range(_M)), trace=trace,
        tmpdir=os.environ.get("KERNEL_TMPDIR") or None,
    )
    LAST_RESULTS = res

    out = np.concatenate([np.asarray(res.results[i]["out"]) for i in range(_M)])
    return out.astype(np.float32)
